# revision 11
# baseline (speedup 1.0000x reference)
"""Trainium2 Bass kernel for nn_IngredientScannerLoss.

Per row (12 coords = 6 (x,y) pairs):
    delta = output - target
    dist_j = sqrt(dx_j^2 + dy_j^2)
    n_j    = (s0_j*dx_j > 0) + (s1_j*dy_j > 0)   (sign-gated count, 0/1/2)
    f(x)   = ((x+1)^1.2 - 1)*2
    t_j    = [dist, f(dist), f(f(dist))][n_j]
    loss   = sum_j t_j

Data-parallel over 8 NeuronCores: rows split 8 x 500_000, each shard
zero-padded to 501_760 = 128*3920 rows so tiles are [128, RT*12].

Engine split per tile:
    GPSIMD: delta = a - b                       (tensor_tensor subtract)
    DVE:    s = dx^2+dy^2 (custom op), n (custom op x6 pair columns),
            d1/d2 affines, predicated selects, row-sum reduce
    ACT:    ln/exp chains (single natural_log_exp table set; sqrt is done
            as exp(0.5*ln s) to avoid table switches)
"""

import numpy as np

import concourse.bacc as bacc
import concourse.bass as bass
import concourse.mybir as mybir
import concourse.tile as tile
from concourse import dve_ops
from concourse.bass_utils import run_bass_kernel_spmd
from concourse.dve_ops import DveOp
from concourse.dve_spec import Spec, Src0, Src1, C0, C1, Zero, _has_src1, lower, sq
from concourse.dve_uop import DveOpSpec

P = 128
COLS = 12
NPAIR = 6
B = 4_000_000
N_CORES = 8
ROWS_VALID = B // N_CORES          # 500_000
RT = 392                           # rows per partition per tile
NT = 10                            # tiles per core
ROWS_PC = P * RT * NT              # 501_760 padded rows per core
LN2 = 0.6931471805599453

# per-coordinate condition signs (see reference _SIGNS)
SIGNS = [1.0, 1.0, 1.0, -1.0, -1.0, -1.0, -1.0, 1.0, 0.0, 1.0, 0.0, -1.0]

F32 = mybir.dt.float32
AF = mybir.ActivationFunctionType
ALU = mybir.AluOpType

# how many pair columns can ever hit n == 2 (pairs 4,5 have s0 == 0 -> n <= 1,
# so the second transform is only needed for pair columns 0..3)
NPAIR2 = 4

# ---------------------------------------------------------------- custom ops


def _register_op(name: str, spec: Spec, subdim: bool = False) -> DveOp:
    for op in dve_ops.OPS:
        if op.name == name:
            return op
    if name not in dve_ops._SUB_OPCODE_FOR_NAME:
        row = max(dve_ops._SUB_OPCODE_FOR_NAME.values()) + 1
        assert row < 0x20, "custom DVE opcode rows exhausted"
        dve_ops._SUB_OPCODE_FOR_NAME[name] = row
    shas = {}
    for ver in ("v3", "v4"):
        try:
            shas[ver] = DveOpSpec(
                name=name,
                opcode=dve_ops.get_dve_sub_opcode(name),
                uops=lower(spec, ver=ver),
                rd1_en=_has_src1(spec),
            ).sha(ver)
        except Exception:
            pass
    op = DveOp(name, spec, subdim, shas)
    dve_ops.OPS.append(op)
    dve_ops.CUSTOM_DVE_SPECS[name] = spec
    return op


# s = in0^2 + in1^2  (in0/in1 = even/odd delta columns)
PAIRDIST = _register_op(
    "ANT_PAIRDIST",
    Spec(
        body=sq(Src0) + sq(Src1),
        reference=lambda in0, in1, s0, s1, imm2: (
            in0.astype(np.float32) ** 2 + in1.astype(np.float32) ** 2
        ),
    ),
)

# n = (in0*s0 > 0) + (in1*s1 > 0)
CGATE = _register_op(
    "ANT_CGATE",
    Spec(
        body=(Src0 * C0 > Zero) + (Src1 * C1 > Zero),
        reference=lambda in0, in1, s0, s1, imm2: (
            ((in0.astype(np.float32) * s0) > 0).astype(np.float32)
            + ((in1.astype(np.float32) * s1) > 0).astype(np.float32)
        ),
    ),
)


# ---------------------------------------------------------------- act tables
# The stock table-load pass resolves Exp -> exp_and_others and
# Ln -> natural_log, reloading ACT tables on every Ln<->Exp switch
# (~2.7us each, ~100us/core total). Restrict ln/exp membership to sets
# that hold BOTH so every activation resolves to
# natural_log_exp_and_others and the load hoists to one per kernel.
# Dict order (and thus act_func_set_id indices) is preserved.

_GAT_REAL = None


def _gat_lnexp(arch):
    global _GAT_REAL
    from concourse.hw_specs import get_activation_tables

    if _GAT_REAL is None:
        _GAT_REAL = get_activation_tables
    tabs = _GAT_REAL(arch)
    out = {}
    for name, funcs in tabs.items():
        fs = set(funcs)
        if not (AF.Ln in fs and AF.Exp in fs):
            fs.discard(AF.Ln)
            fs.discard(AF.Exp)
        out[name] = fs
    return out


def _patch_act_tables():
    if bacc.get_activation_tables is not _gat_lnexp:
        global _GAT_REAL
        _GAT_REAL = bacc.get_activation_tables
        bacc.get_activation_tables = _gat_lnexp


# ---------------------------------------------------------------- bass build


def build_nc(rt: int = RT, nt: int = NT):
    """Build the single-core SPMD program for [P*rt*nt, 12] inputs."""
    _patch_act_tables()
    rows = P * rt * nt
    nc = bacc.Bacc("TRN2", debug=False, target_bir_lowering=False,
                   num_devices=N_CORES)
    # activation biases need registered const APs (only 0.0/1.0 ship)
    for cv in (-1.0, LN2):
        if (F32, cv) not in nc.const_aps.aps:
            ct = nc.alloc_sbuf_tensor(f"const-f32-{cv}", [P, 1], F32)
            nc.gpsimd.memset(ct.ap(), cv)
            nc.const_aps.aps[(F32, cv)] = ct.ap()
    nc.all_engine_barrier()
    a = nc.dram_tensor("output", [rows, COLS], F32, kind="ExternalInput").ap()
    b = nc.dram_tensor("target", [rows, COLS], F32, kind="ExternalInput").ap()
    o = nc.dram_tensor("loss", [rows], F32, kind="ExternalOutput").ap()

    a3 = a.rearrange("(n p r) m -> n p (r m)", p=P, r=rt)
    b3 = b.rearrange("(n p r) m -> n p (r m)", p=P, r=rt)
    o3 = o.rearrange("(n p r) -> n p r", p=P, r=rt)

    I32 = mybir.dt.int32
    with tile.TileContext(nc) as tc:
        with tc.tile_pool(name="sb", bufs=2) as pool:
            for i in range(nt):
                w4 = rt * NPAIR2
                w6 = rt * NPAIR
                ta = pool.tile([P, rt * COLS], F32, tag="ta")
                nc.sync.dma_start(out=ta[:], in_=a3[i])
                tb = pool.tile([P, rt * COLS], F32, tag="tb")
                nc.sync.dma_start(out=tb[:], in_=b3[i])

                # delta split into even/odd coordinate blocks, pair-major:
                # delta[:, comp*6rt + j*rt + r] = a[12r+2j+comp]-b[...]
                delta = pool.tile([P, rt * COLS], F32, tag="delta")
                d4 = delta[:].rearrange("p (two j r) -> p two j r",
                                        two=2, j=NPAIR)
                a4 = ta[:].rearrange("p (r j two) -> p two j r",
                                     two=2, j=NPAIR)
                b4 = tb[:].rearrange("p (r j two) -> p two j r",
                                     two=2, j=NPAIR)
                nc.gpsimd.tensor_tensor(d4, a4, b4, ALU.subtract)
                dE = delta[:, 0:w6]
                dO = delta[:, w6:2 * w6]

                # s = dx^2 + dy^2, [P, 6*rt] pair-major (contiguous ins)
                s = pool.tile([P, w6], F32, tag="s")
                nc.vector._custom_dve(PAIRDIST, out=s[:], in0=dE, in1=dO)

                # n gates, [P, 6*rt] pair-major. Pairs 4,5 have s0==0 so
                # n = (s1*dy > 0): single-src tensor_scalar (2x mode).
                n = pool.tile([P, w6], F32, tag="n")
                for j in range(NPAIR):
                    xs = slice(j * rt, (j + 1) * rt)
                    if SIGNS[2 * j] != 0.0:
                        nc.vector._custom_dve(
                            CGATE,
                            out=n[:, xs],
                            in0=delta[:, j * rt:(j + 1) * rt],
                            in1=delta[:, w6 + j * rt:w6 + (j + 1) * rt],
                            s0=SIGNS[2 * j],
                            s1=SIGNS[2 * j + 1],
                        )
                    else:
                        op = ALU.is_gt if SIGNS[2 * j + 1] > 0 else ALU.is_lt
                        nc.vector.tensor_scalar(
                            n[:, xs], delta[:, w6 + j * rt:w6 + (j + 1) * rt],
                            0.0, None, op)

                # ACT chain, one table set (ln+exp):
                #   lt  = ln(s)                  (in-place on s)
                #   res = exp(0.5*lt) = dist
                #   t   = ln(res + 1)
                #   W0  = exp(1.2*t + ln2) = 2u  (in-place on t)
                #   t2  = ln(W0 - 1) = ln(2u-1)
                #   W1  = exp(1.2*t2 + ln2) = 2v (in-place on t2)
                nc.scalar.activation(s[:], s[:], AF.Ln)
                res = pool.tile([P, w6], F32, tag="res")
                nc.scalar.activation(res[:], s[:], AF.Exp, scale=0.5)
                t = pool.tile([P, w6], F32, tag="t")
                nc.scalar.activation(t[:], res[:], AF.Ln, bias=1.0)
                nc.scalar.activation(t[:], t[:], AF.Exp, scale=1.2, bias=LN2)
                t2 = pool.tile([P, w4], F32, tag="t2")
                nc.scalar.activation(t2[:], t[:, 0:w4], AF.Ln, bias=-1.0)
                nc.scalar.activation(t2[:], t2[:], AF.Exp, scale=1.2, bias=LN2)

                # d1 = W0 - 2 = 2u - 2 (in-place), d2 = W1 - 2 (in-place)
                nc.vector.tensor_scalar(t[:], t[:], 2.0, None, ALU.subtract)
                nc.vector.tensor_scalar(t2[:], t2[:], 2.0, None, ALU.subtract)

                # res (= dist) overwritten by d1 where n>=1, d2 where n>=2.
                # CopyPredicated wants an integer mask; fp32 {0.,1.,2.}
                # bitcast to int32 is nonzero exactly where the float is.
                nc.vector.copy_predicated(res[:], n[:].bitcast(I32), t[:])
                # m2 = relu(n-1), in-place on n prefix (after cp1 read n)
                nc.vector.tensor_scalar(n[:, 0:w4], n[:, 0:w4], 1.0, 0.0,
                                        ALU.subtract, ALU.max)
                nc.vector.copy_predicated(res[:, 0:w4],
                                          n[:, 0:w4].bitcast(I32), t2[:])

                # row sums via contiguous add tree (pair order irrelevant)
                w3 = rt * 3
                nc.vector.tensor_tensor(res[:, 0:w3], res[:, 0:w3],
                                        res[:, w3:2 * w3], ALU.add)
                nc.vector.tensor_tensor(res[:, 0:rt], res[:, 0:rt],
                                        res[:, rt:2 * rt], ALU.add)
                ot = pool.tile([P, rt], F32, tag="ot")
                nc.vector.tensor_tensor(ot[:], res[:, 0:rt],
                                        res[:, 2 * rt:w3], ALU.add)
                nc.sync.dma_start(out=o3[i], in_=ot[:])
    nc.compile()
    return nc


_NC_CACHE: dict = {}


def _get_nc(rt: int = RT, nt: int = NT):
    key = (rt, nt)
    if key not in _NC_CACHE:
        _NC_CACHE[key] = build_nc(rt, nt)
    return _NC_CACHE[key]


# ---------------------------------------------------------------- entrypoint


def kernel(output, target):
    a = np.asarray(output, dtype=np.float32)
    b = np.asarray(target, dtype=np.float32)
    assert a.shape == (B, COLS) and b.shape == (B, COLS)

    a_sh = np.zeros((N_CORES, ROWS_PC, COLS), dtype=np.float32)
    b_sh = np.zeros((N_CORES, ROWS_PC, COLS), dtype=np.float32)
    a_sh[:, :ROWS_VALID, :] = a.reshape(N_CORES, ROWS_VALID, COLS)
    b_sh[:, :ROWS_VALID, :] = b.reshape(N_CORES, ROWS_VALID, COLS)

    nc = _get_nc()
    in_maps = [
        {"output": a_sh[c], "target": b_sh[c]} for c in range(N_CORES)
    ]
    r = run_bass_kernel_spmd(nc, in_maps, list(range(N_CORES)))
    out = np.empty((N_CORES, ROWS_VALID), dtype=np.float32)
    for c in range(N_CORES):
        out[c] = r.results[c]["loss"][:ROWS_VALID]
    return out.reshape(B)
